# revision 12
# baseline (speedup 1.0000x reference)
"""Trainium2 Bass kernel for MultiHeadDeformableAttention2D.

Strategy (8 cores, SPMD): data-parallel over (batch n, row-half). Core c
handles n = c//2 and 512 query rows. The deformable sampling is cast as a
sparse matrix multiply: out_pre = S @ q where S[l, cell] accumulates
attn*bilinear weights. S is built with per-partition GPSIMD local_scatter
into 4 corner-class planes; same-base-cell duplicates within a class are
merged exactly beforehand via a 32x32 tap-equality matrix on the vector
engine. The contraction with q and the output projection W_out run on the
tensor engine in fp16 with fp32 accumulation; the 4 planes are folded into
the contraction (PSUM accumulation) so no plane-combine pass is needed.
"""

import sys

sys.path.insert(0, "/opt/trn_rl_repo")

import numpy as np

import concourse.bass as bass  # noqa: F401
import concourse.tile as tile
from concourse import bacc, mybir
from concourse.bass_interp import get_hw_module
from concourse.bass_utils import run_bass_kernel_spmd
from concourse.masks import make_identity

P = 128
L = 1024          # spatial cells (32*32)
E = 256           # embed dim
NT = 32           # taps = heads(8) * points(4)
ROWS = 512        # rows per core
N_CORES = 8
N_PLANES = 4      # corner-class planes folded into the PE contraction
MAGIC = 12582912.0  # 1.5*2^23; (x + MAGIC) - MAGIC rounds x to nearest int

f32 = mybir.dt.float32
f16 = mybir.dt.float16
i16 = mybir.dt.int16
i32 = mybir.dt.int32
Alu = mybir.AluOpType
Act = mybir.ActivationFunctionType


def _emit(nc, tc, d, sx):
    """Emit one full kernel pass. d: dict of dram tensors, sx: name suffix."""
    with (
        tc.tile_pool(name=f"const{sx}", bufs=1) as cp,
        tc.tile_pool(name=f"work{sx}", bufs=3) as wp,
        tc.tile_pool(name=f"psA{sx}", bufs=2, space="PSUM") as psA,
        tc.tile_pool(name=f"psB{sx}", bufs=2, space="PSUM") as psB,
    ):
        # ---------------- constants ----------------
        ident = cp.tile([P, P], f32, name=f"ident{sx}")
        make_identity(nc, ident[:])
        ones1 = cp.tile([1, P], f32, name=f"ones1{sx}")
        nc.vector.memset(ones1[:], 1.0)
        ones1h = cp.tile([1, P], f16, name=f"ones1h{sx}")
        nc.vector.memset(ones1h[:], 1.0)

        # strict-lower mask lt16[t*32+s] = 1.0 iff s < t
        lt_i = cp.tile([P, NT * NT], i32, name=f"lt_i{sx}")
        nc.gpsimd.iota(lt_i[:], pattern=[[-1, NT], [1, NT]], base=0,
                       channel_multiplier=0)
        lt16 = cp.tile([P, NT * NT], f16, name=f"lt16{sx}")
        nc.vector.tensor_scalar(lt16[:], lt_i[:], -1.0, None, Alu.is_le)

        # per-class cell offsets (+1 for the idx trick), class-major [c*32+t]
        off4 = cp.tile([P, 4 * NT], f32, name=f"off4{sx}")
        for c, off in enumerate((1.0, 2.0, 33.0, 34.0)):
            nc.vector.memset(off4[:, c * NT:(c + 1) * NT], off)

        # ---------------- loads ----------------
        qn3 = d["qn"].ap().rearrange("(t p) e -> t p e", p=P)
        qloc3 = d["qloc"].ap().rearrange("(t p) e -> t p e", p=P)
        refs3 = d["refs"].ap().rearrange("(t p) c -> t p c", p=P)

        q16 = []
        for i in range(8):
            qt = wp.tile([P, E], f32, tag="qload", name=f"qload{sx}_{i}")
            nc.sync.dma_start(qt[:], qn3[i])
            qh = cp.tile([P, E], f16, name=f"q16{sx}_{i}")
            nc.scalar.copy(qh[:], qt[:])
            q16.append(qh)

        qloc = []
        for i in range(4):
            t = cp.tile([P, E], f32, name=f"qloc{sx}_{i}")
            nc.sync.dma_start(t[:], qloc3[i])
            qloc.append(t)

        refs = []
        for i in range(4):
            t = cp.tile([P, 2], f32, name=f"refs{sx}_{i}")
            nc.sync.dma_start(t[:], refs3[i])
            refs.append(t)

        wcat3 = d["wcat"].ap().rearrange("(t p) c -> t p c", p=P)
        wcat = []
        for i in range(2):
            t = cp.tile([P, 96], f32, name=f"wcat{sx}_{i}")
            nc.sync.dma_start(t[:], wcat3[i])
            wcat.append(t)

        wout3 = d["wout"].ap().rearrange("(t p) c -> t p c", p=P)
        wout16 = []
        for i in range(2):
            t = wp.tile([P, E], f32, tag="wo_load", name=f"wo_load{sx}_{i}")
            nc.sync.dma_start(t[:], wout3[i])
            th = cp.tile([P, E], f16, name=f"wout16{sx}_{i}")
            nc.scalar.copy(th[:], t[:])
            wout16.append(th)

        bcat = cp.tile([1, 96], f32, name=f"bcat{sx}")
        nc.sync.dma_start(bcat[:], d["bcat"].ap()[:])
        bout = cp.tile([1, E], f32, name=f"bout{sx}")
        nc.sync.dma_start(bout[:], d["bout"].ap()[:])
        bout16 = cp.tile([1, E], f16, name=f"bout16{sx}")
        nc.scalar.copy(bout16[:], bout[:])

        # ---------------- q_loc^T (for projections) ----------------
        qlocT = [cp.tile([P, ROWS], f32, name=f"qlocT{sx}_{eh}") for eh in range(2)]
        for lb in range(4):
            for eh in range(2):
                tp = psA.tile([P, P], f32, space="PSUM", tag="tp_ps",
                              name=f"tp_ps{sx}_{lb}_{eh}")
                nc.tensor.transpose(out=tp[:], in_=qloc[lb][:, eh * P:(eh + 1) * P],
                                    identity=ident[:])
                nc.scalar.copy(qlocT[eh][:, lb * P:(lb + 1) * P], tp[:])

        # ---------------- S^T staging (one buffer per corner plane) --------
        stall = [cp.tile([P, 8 * ROWS], f16, name=f"stall{sx}_{pp}")
                 for pp in range(N_PLANES)]

        # ---------------- per-rowtile pipeline ----------------
        for lb in range(4):
            proj_ps = psA.tile([P, 96], f32, space="PSUM", tag="proj_ps",
                               name=f"proj_ps{sx}_{lb}")
            nc.tensor.matmul(proj_ps[:], lhsT=qlocT[0][:, lb * P:(lb + 1) * P],
                             rhs=wcat[0][:], start=True, stop=False)
            nc.tensor.matmul(proj_ps[:], lhsT=qlocT[1][:, lb * P:(lb + 1) * P],
                             rhs=wcat[1][:], start=False, stop=False)
            nc.tensor.matmul(proj_ps[:], lhsT=ones1[:], rhs=bcat[:],
                             start=False, stop=True)
            proj = wp.tile([P, 96], f32, tag="proj", name=f"proj{sx}_{lb}")
            nc.scalar.copy(proj[:], proj_ps[:])

            refx = refs[lb][:, 0:1]
            refy = refs[lb][:, 1:2]

            # pos: x in cols 0:32, y in cols 32:64
            pos = wp.tile([P, 2 * NT], f32, tag="pos", name=f"pos{sx}_{lb}")
            nc.vector.tensor_scalar(pos[:, 0:NT], proj[:, 0:64:2], refx, 32.0,
                                    Alu.add, Alu.mult)
            nc.vector.tensor_scalar(pos[:, NT:2 * NT], proj[:, 1:64:2], refy, 32.0,
                                    Alu.add, Alu.mult)

            # floors (batched x|y): f = rni(pos - 1.0)  [= floor(pos - 0.5)]
            fl = wp.tile([P, 2 * NT], f32, tag="fl", name=f"fl{sx}_{lb}")
            nc.vector.tensor_scalar(fl[:], pos[:], 1.0 - MAGIC, None, Alu.subtract)
            nc.vector.tensor_scalar(fl[:], fl[:], MAGIC, None, Alu.subtract)
            fx = fl[:, 0:NT]
            fy = fl[:, NT:2 * NT]

            # fracs: dd = pos - f; w1 = dd-0.5, w0 = 1.5-dd  (batched x|y)
            dd = wp.tile([P, 2 * NT], f32, tag="dd", name=f"dd{sx}_{lb}")
            nc.vector.tensor_sub(dd[:], pos[:], fl[:])
            w1 = wp.tile([P, 2 * NT], f32, tag="w1", name=f"w1{sx}_{lb}")
            nc.vector.tensor_scalar(w1[:], dd[:], 0.5, None, Alu.subtract)
            w0 = wp.tile([P, 2 * NT], f32, tag="w0", name=f"w0{sx}_{lb}")
            nc.vector.tensor_scalar(w0[:], dd[:], -1.0, 1.5, Alu.mult, Alu.add)
            # wxpair = [wx0 | wx1], wypair = [wy0 | wy1]
            wxpair = wp.tile([P, 2 * NT], f32, tag="wxp", name=f"wxp{sx}_{lb}")
            nc.scalar.copy(wxpair[:, 0:NT], w0[:, 0:NT])
            nc.scalar.copy(wxpair[:, NT:2 * NT], w1[:, 0:NT])
            wypair = wp.tile([P, 2 * NT], f32, tag="wyp", name=f"wyp{sx}_{lb}")
            nc.scalar.copy(wypair[:, 0:NT], w0[:, NT:2 * NT])
            nc.scalar.copy(wypair[:, NT:2 * NT], w1[:, NT:2 * NT])

            # attention softmax over points, /8 folded into reciprocal
            ex = wp.tile([P, NT], f32, tag="ex", name=f"ex{sx}_{lb}")
            nc.scalar.activation(ex[:], proj[:, 64:96], Act.Exp)
            r1 = wp.tile([P, 16], f32, tag="r1", name=f"r1{sx}_{lb}")
            nc.vector.tensor_add(r1[:], ex[:, 0:32:2], ex[:, 1:32:2])
            s4 = wp.tile([P, 8], f32, tag="s4", name=f"s4{sx}_{lb}")
            nc.vector.tensor_add(s4[:], r1[:, 0:16:2], r1[:, 1:16:2])
            rcp = wp.tile([P, 8], f32, tag="rcp", name=f"rcp{sx}_{lb}")
            nc.vector.reciprocal(rcp[:], s4[:])
            nc.vector.tensor_scalar(rcp[:], rcp[:], 0.125, None, Alu.mult)
            a8 = wp.tile([P, NT], f32, tag="a8", name=f"a8{sx}_{lb}")
            nc.vector.tensor_tensor(
                a8[:].rearrange("p (h q) -> p h q", q=4),
                ex[:].rearrange("p (h q) -> p h q", q=4),
                rcp[:].unsqueeze(2).broadcast_to([P, 8, 4]),
                op=Alu.mult)

            # axpair = wxpair * attn/8 (attn replicated over the 2 halves)
            axpair = wp.tile([P, 2 * NT], f32, tag="axp", name=f"axp{sx}_{lb}")
            nc.vector.tensor_tensor(
                axpair[:].rearrange("p (h t) -> p h t", t=NT),
                wxpair[:].rearrange("p (h t) -> p h t", t=NT),
                a8[:].unsqueeze(1).broadcast_to([P, 2, NT]),
                op=Alu.mult)

            # corner weights, class-major [c*32+t], c = cy*2+cx
            wit = wp.tile([P, 4 * NT], f32, tag="wit", name=f"wit{sx}_{lb}")
            nc.vector.tensor_tensor(
                wit[:].rearrange("p (cy cx t) -> p cy cx t", cy=2, cx=2),
                axpair[:].rearrange("p (cx t) -> p cx t", cx=2)
                    .unsqueeze(1).broadcast_to([P, 2, 2, NT]),
                wypair[:].rearrange("p (cy t) -> p cy t", cy=2)
                    .unsqueeze(2).broadcast_to([P, 2, 2, NT]),
                op=Alu.mult)
            wit16 = wp.tile([P, 4 * NT], f16, tag="wit16", name=f"wit16{sx}_{lb}")
            nc.scalar.copy(wit16[:], wit[:])

            # cell = 32*fy + fx ; alias-free key bkey = cell + 96*fy
            cell = wp.tile([P, NT], f32, tag="cell", name=f"cell{sx}_{lb}")
            nc.vector.tensor_scalar(cell[:], fy, 32.0, None, Alu.mult)
            nc.vector.tensor_tensor(cell[:], cell[:], fx, op=Alu.add)
            bkey = wp.tile([P, NT], f32, tag="bkey", name=f"bkey{sx}_{lb}")
            nc.vector.tensor_scalar(bkey[:], fy, 96.0, None, Alu.mult)
            nc.vector.tensor_tensor(bkey[:], bkey[:], cell[:], op=Alu.add)

            # validity (batched x|y): v0 = in [0,31], v1 = in [-1,30]
            v0 = wp.tile([P, 2 * NT], f32, tag="v0", name=f"v0{sx}_{lb}")
            nc.vector.tensor_scalar(v0[:], fl[:], 0.0, None, Alu.is_ge)
            vtmp = wp.tile([P, 2 * NT], f32, tag="vtmp", name=f"vtmp{sx}_{lb}")
            nc.vector.tensor_scalar(vtmp[:], fl[:], 31.0, None, Alu.is_le)
            nc.vector.tensor_tensor(v0[:], v0[:], vtmp[:], op=Alu.mult)
            v1 = wp.tile([P, 2 * NT], f32, tag="v1", name=f"v1{sx}_{lb}")
            nc.vector.tensor_scalar(v1[:], fl[:], -1.0, None, Alu.is_ge)
            nc.vector.tensor_scalar(vtmp[:], fl[:], 30.0, None, Alu.is_le)
            nc.vector.tensor_tensor(v1[:], v1[:], vtmp[:], op=Alu.mult)
            # vxpair = [vx0 | vx1], vypair = [vy0 | vy1]
            vxpair = wp.tile([P, 2 * NT], f32, tag="vxp", name=f"vxp{sx}_{lb}")
            nc.scalar.copy(vxpair[:, 0:NT], v0[:, 0:NT])
            nc.scalar.copy(vxpair[:, NT:2 * NT], v1[:, 0:NT])
            vypair = wp.tile([P, 2 * NT], f32, tag="vyp", name=f"vyp{sx}_{lb}")
            nc.scalar.copy(vypair[:, 0:NT], v0[:, NT:2 * NT])
            nc.scalar.copy(vypair[:, NT:2 * NT], v1[:, NT:2 * NT])

            # --- tap equality matrix (f16 out) + keep mask ---
            eq16 = wp.tile([P, NT * NT], f16, tag="eq16", name=f"eq16{sx}_{lb}")
            nc.vector.tensor_tensor(
                eq16[:].rearrange("p (t s) -> p t s", s=NT),
                bkey[:].unsqueeze(2).broadcast_to([P, NT, NT]),
                bkey[:].unsqueeze(1).broadcast_to([P, NT, NT]),
                op=Alu.is_equal)
            dupp = wp.tile([P, NT * NT], f16, tag="dupp", name=f"dupp{sx}_{lb}")
            nc.vector.tensor_tensor(dupp[:], eq16[:], lt16[:], op=Alu.mult)
            dupf = wp.tile([P, NT], f16, tag="dupf", name=f"dupf{sx}_{lb}")
            with nc.allow_low_precision(reason="0/1 flags, fp32 internal"):
                nc.vector.tensor_reduce(
                    dupf[:].unsqueeze(2),
                    dupp[:].rearrange("p (t s) -> p t s", s=NT),
                    op=Alu.max, axis=mybir.AxisListType.X)
            keep = wp.tile([P, NT], f32, tag="keep", name=f"keep{sx}_{lb}")
            nc.vector.tensor_scalar(keep[:], dupf[:], -1.0, 1.0, Alu.mult, Alu.add)

            # --- merged weights, all 4 classes in one TT + one reduce ---
            prod4 = wp.tile([P, 4 * NT * NT], f16, tag="prod4",
                            name=f"prod4{sx}_{lb}")
            nc.vector.tensor_tensor(
                prod4[:].rearrange("p (c t s) -> p c t s", c=4, t=NT),
                eq16[:].rearrange("p (t s) -> p t s", s=NT)
                    .unsqueeze(1).broadcast_to([P, 4, NT, NT]),
                wit16[:].rearrange("p (c s) -> p c s", c=4)
                    .unsqueeze(2).broadcast_to([P, 4, NT, NT]),
                op=Alu.mult)
            mw = wp.tile([P, 4 * NT], f16, tag="mw", name=f"mw{sx}_{lb}")
            with nc.allow_low_precision(reason="<=6 small f16 addends"):
                nc.vector.tensor_reduce(
                    mw[:].unsqueeze(2),
                    prod4[:].rearrange("p (ct s) -> p ct s", s=NT),
                    op=Alu.add, axis=mybir.AxisListType.X)

            # --- scatter indices: idx = keep*vx*vy*(cell+off_c+...) - 1 ---
            km4 = wp.tile([P, 4 * NT], f32, tag="km4", name=f"km4{sx}_{lb}")
            nc.vector.tensor_tensor(
                km4[:].rearrange("p (cy cx t) -> p cy cx t", cy=2, cx=2),
                vxpair[:].rearrange("p (cx t) -> p cx t", cx=2)
                    .unsqueeze(1).broadcast_to([P, 2, 2, NT]),
                vypair[:].rearrange("p (cy t) -> p cy t", cy=2)
                    .unsqueeze(2).broadcast_to([P, 2, 2, NT]),
                op=Alu.mult)
            nc.vector.tensor_tensor(
                km4[:].rearrange("p (c t) -> p c t", c=4),
                km4[:].rearrange("p (c t) -> p c t", c=4),
                keep[:].unsqueeze(1).broadcast_to([P, 4, NT]),
                op=Alu.mult)
            cell4 = wp.tile([P, 4 * NT], f32, tag="cell4", name=f"cell4{sx}_{lb}")
            nc.vector.tensor_tensor(
                cell4[:].rearrange("p (c t) -> p c t", c=4),
                cell[:].unsqueeze(1).broadcast_to([P, 4, NT]),
                off4[:].rearrange("p (c t) -> p c t", c=4),
                op=Alu.add)
            nc.vector.tensor_tensor(cell4[:], cell4[:], km4[:], op=Alu.mult)
            nc.vector.tensor_scalar(cell4[:], cell4[:], 1.0, None, Alu.subtract)
            idx16 = wp.tile([P, 4 * NT], i16, tag="idx16", name=f"idx16{sx}_{lb}")
            nc.vector.tensor_copy(idx16[:], cell4[:])

            # --- scatters: one fp16 plane per corner class ---
            for c in range(N_PLANES):
                pl = wp.tile([P, L], f16, tag=f"plane{c}",
                             name=f"plane{sx}_{lb}_{c}")
                nc.gpsimd.local_scatter(pl[:], mw[:, c * NT:(c + 1) * NT],
                                        idx16[:, c * NT:(c + 1) * NT],
                                        channels=P, num_elems=L, num_idxs=NT)
                dst = stall[c][:, :].rearrange("p (kt l) -> p kt l", l=ROWS)
                dst = dst[:, :, lb * P:(lb + 1) * P]
                nc.sync.dma_start_transpose(out=dst, in_=pl[:])

        # ---------------- sampling matmul (planes folded into K) ----------
        # Per l-block accumulation groups so each rowtile's matmuls can start
        # as soon as its transposed planes land, instead of waiting for all.
        outT16 = [cp.tile([P, ROWS], f16, name=f"outT16{sx}_{eh}")
                  for eh in range(2)]
        for lb in range(4):
            for eh in range(2):
                ps = psB.tile([P, P], f32, space="PSUM", tag="outT_ps",
                              name=f"outT_ps{sx}_{lb}_{eh}")
                first = True
                for pp in range(N_PLANES):
                    st3 = stall[pp][:, :].rearrange("p (kt l) -> p kt l", l=ROWS)
                    for kt in range(8):
                        nc.tensor.matmul(ps[:],
                                         lhsT=q16[kt][:, eh * P:(eh + 1) * P],
                                         rhs=st3[:, kt, lb * P:(lb + 1) * P],
                                         start=first,
                                         stop=(pp == N_PLANES - 1 and kt == 7))
                        first = False
                nc.vector.tensor_copy(outT16[eh][:, lb * P:(lb + 1) * P], ps[:])

        # ---------------- final projection ----------------
        out3 = d["out"].ap().rearrange("(t p) e -> t p e", p=P)
        for lb in range(4):
            fin = psA.tile([P, E], f32, space="PSUM", tag="fin_ps",
                           name=f"fin_ps{sx}_{lb}")
            nc.tensor.matmul(fin[:], lhsT=outT16[0][:, lb * P:(lb + 1) * P],
                             rhs=wout16[0][:], start=True, stop=False)
            nc.tensor.matmul(fin[:], lhsT=outT16[1][:, lb * P:(lb + 1) * P],
                             rhs=wout16[1][:], start=False, stop=False)
            nc.tensor.matmul(fin[:], lhsT=ones1h[:], rhs=bout16[:],
                             start=False, stop=True)
            osb = wp.tile([P, E], f32, tag="osb", name=f"osb{sx}_{lb}")
            nc.scalar.copy(osb[:], fin[:])
            nc.sync.dma_start(out3[lb], osb[:])


def build_program(repeat=1, strip=True):
    nc = bacc.Bacc("TRN2", target_bir_lowering=False, debug=False)

    d = {
        "qn": nc.dram_tensor("qn", [L, E], f32, kind="ExternalInput"),
        "qloc": nc.dram_tensor("qloc", [ROWS, E], f32, kind="ExternalInput"),
        "refs": nc.dram_tensor("refs", [ROWS, 2], f32, kind="ExternalInput"),
        "wcat": nc.dram_tensor("wcat", [E, 96], f32, kind="ExternalInput"),
        "wout": nc.dram_tensor("wout", [E, E], f32, kind="ExternalInput"),
        "bcat": nc.dram_tensor("bcat", [1, 96], f32, kind="ExternalInput"),
        "bout": nc.dram_tensor("bout", [1, E], f32, kind="ExternalInput"),
        "out": nc.dram_tensor("out", [ROWS, E], f32, kind="ExternalOutput"),
    }

    with tile.TileContext(nc) as tc:
        if repeat == 1:
            _emit(nc, tc, d, "")
        else:
            with tc.For_i(0, repeat, 1):
                _emit(nc, tc, d, "")

    nc.compile()
    if strip:
        nc.m = get_hw_module(nc.m)
    return nc


_NC = None


def _get_nc():
    global _NC
    if _NC is None:
        _NC = build_program()
    return _NC


def make_in_maps(inputs):
    query = np.asarray(inputs["query"], np.float32)
    refp = np.asarray(inputs["reference_points"], np.float32)
    W_off = np.asarray(inputs["W_off"], np.float32)
    b_off = np.asarray(inputs["b_off"], np.float32)
    W_attn = np.asarray(inputs["W_attn"], np.float32)
    b_attn = np.asarray(inputs["b_attn"], np.float32)
    W_out = np.asarray(inputs["W_out"], np.float32)
    b_out = np.asarray(inputs["b_out"], np.float32)

    N = query.shape[0]
    q = query.reshape(N, L, E)
    wcat = np.ascontiguousarray(np.concatenate([W_off, W_attn], axis=1))
    bcat = np.ascontiguousarray(np.concatenate([b_off, b_attn])[None, :])
    bout = np.ascontiguousarray(b_out[None, :])

    in_maps = []
    for c in range(N_CORES):
        n, half = c // 2, c % 2
        lo = half * ROWS
        in_maps.append({
            "qn": np.ascontiguousarray(q[n]),
            "qloc": np.ascontiguousarray(q[n, lo:lo + ROWS]),
            "refs": np.ascontiguousarray(refp[n, lo:lo + ROWS]),
            "wcat": wcat,
            "wout": np.ascontiguousarray(W_out),
            "bcat": bcat,
            "bout": bout,
        })
    return in_maps


def kernel(**inputs):
    nc = _get_nc()
    in_maps = make_in_maps(inputs)
    res = run_bass_kernel_spmd(nc, in_maps, list(range(N_CORES)))
    N = np.asarray(inputs["query"]).shape[0]
    out = np.empty((N, L, E), np.float32)
    for c in range(N_CORES):
        n, half = c // 2, c % 2
        out[n, half * ROWS:(half + 1) * ROWS] = res.results[c]["out"]
    return out.reshape(N, 32, 32, E)


# revision 13
# speedup vs baseline: 1.3866x; 1.3866x over previous
"""Trainium2 Bass kernel for MultiHeadDeformableAttention2D.

Strategy (8 cores, SPMD): data-parallel over (batch n, row-half). Core c
handles n = c//2 and 512 query rows. The deformable sampling is cast as a
sparse matrix multiply: out_pre = S @ q where S[l, cell] accumulates
attn*bilinear weights. S is built with per-partition GPSIMD local_scatter
into 4 corner-class planes; same-base-cell duplicates within a class are
merged exactly beforehand via a 32x32 tap-equality matrix on the vector
engine. The contraction with q and the output projection W_out run on the
tensor engine in fp16 with fp32 accumulation; the 4 planes are folded into
the contraction (PSUM accumulation) so no plane-combine pass is needed.
"""

import sys

sys.path.insert(0, "/opt/trn_rl_repo")

import numpy as np

import concourse.bass as bass  # noqa: F401
import concourse.tile as tile
from concourse import bacc, mybir
from concourse.bass_interp import get_hw_module
from concourse.bass_utils import run_bass_kernel_spmd
from concourse.masks import make_identity

P = 128
L = 1024          # spatial cells (32*32)
E = 256           # embed dim
NT = 32           # taps = heads(8) * points(4)
ROWS = 512        # rows per core
N_CORES = 8
N_PLANES = 4      # corner-class planes folded into the PE contraction
MAGIC = 12582912.0  # 1.5*2^23; (x + MAGIC) - MAGIC rounds x to nearest int

f32 = mybir.dt.float32
f16 = mybir.dt.float16
i16 = mybir.dt.int16
i32 = mybir.dt.int32
Alu = mybir.AluOpType
Act = mybir.ActivationFunctionType


def _emit(nc, tc, d, sx):
    """Emit one full kernel pass. d: dict of dram tensors, sx: name suffix."""
    with (
        tc.tile_pool(name=f"const{sx}", bufs=1) as cp,
        tc.tile_pool(name=f"work{sx}", bufs=3) as wp,
        tc.tile_pool(name=f"psA{sx}", bufs=2, space="PSUM") as psA,
        tc.tile_pool(name=f"psB{sx}", bufs=2, space="PSUM") as psB,
    ):
        # ---------------- constants ----------------
        ident = cp.tile([P, P], f32, name=f"ident{sx}")
        make_identity(nc, ident[:])
        ones1 = cp.tile([1, P], f32, name=f"ones1{sx}")
        nc.vector.memset(ones1[:], 1.0)
        ones1h = cp.tile([1, P], f16, name=f"ones1h{sx}")
        nc.vector.memset(ones1h[:], 1.0)

        # strict-lower mask lt16[t*32+s] = 1.0 iff s < t
        lt_i = cp.tile([P, NT * NT], i32, name=f"lt_i{sx}")
        nc.gpsimd.iota(lt_i[:], pattern=[[-1, NT], [1, NT]], base=0,
                       channel_multiplier=0)
        lt16 = cp.tile([P, NT * NT], f16, name=f"lt16{sx}")
        nc.vector.tensor_scalar(lt16[:], lt_i[:], -1.0, None, Alu.is_le)

        # per-class cell offsets (+1 for the idx trick), class-major [c*32+t]
        off4 = cp.tile([P, 4 * NT], f32, name=f"off4{sx}")
        for c, off in enumerate((1.0, 2.0, 33.0, 34.0)):
            nc.vector.memset(off4[:, c * NT:(c + 1) * NT], off)

        # ---------------- loads ----------------
        qn3 = d["qn"].ap().rearrange("(t p) e -> t p e", p=P)
        qloc3 = d["qloc"].ap().rearrange("(t p) e -> t p e", p=P)
        refs3 = d["refs"].ap().rearrange("(t p) c -> t p c", p=P)

        q16 = []
        for i in range(8):
            qt = wp.tile([P, E], f32, tag="qload", name=f"qload{sx}_{i}")
            nc.sync.dma_start(qt[:], qn3[i])
            qh = cp.tile([P, E], f16, name=f"q16{sx}_{i}")
            nc.scalar.copy(qh[:], qt[:])
            q16.append(qh)

        qloc = []
        for i in range(4):
            t = cp.tile([P, E], f32, name=f"qloc{sx}_{i}")
            nc.sync.dma_start(t[:], qloc3[i])
            qloc.append(t)

        refs = []
        for i in range(4):
            t = cp.tile([P, 2], f32, name=f"refs{sx}_{i}")
            nc.sync.dma_start(t[:], refs3[i])
            refs.append(t)

        wcat3 = d["wcat"].ap().rearrange("(t p) c -> t p c", p=P)
        wcat = []
        for i in range(2):
            t = cp.tile([P, 96], f32, name=f"wcat{sx}_{i}")
            nc.sync.dma_start(t[:], wcat3[i])
            wcat.append(t)

        wout3 = d["wout"].ap().rearrange("(t p) c -> t p c", p=P)
        wout16 = []
        for i in range(2):
            t = wp.tile([P, E], f32, tag="wo_load", name=f"wo_load{sx}_{i}")
            nc.sync.dma_start(t[:], wout3[i])
            th = cp.tile([P, E], f16, name=f"wout16{sx}_{i}")
            nc.scalar.copy(th[:], t[:])
            wout16.append(th)

        bcat = cp.tile([1, 96], f32, name=f"bcat{sx}")
        nc.sync.dma_start(bcat[:], d["bcat"].ap()[:])
        bout = cp.tile([1, E], f32, name=f"bout{sx}")
        nc.sync.dma_start(bout[:], d["bout"].ap()[:])
        bout16 = cp.tile([1, E], f16, name=f"bout16{sx}")
        nc.scalar.copy(bout16[:], bout[:])

        # ---------------- q_loc^T (for projections) ----------------
        qlocT = [cp.tile([P, ROWS], f32, name=f"qlocT{sx}_{eh}") for eh in range(2)]
        for lb in range(4):
            for eh in range(2):
                tp = psA.tile([P, P], f32, space="PSUM", tag="tp_ps",
                              name=f"tp_ps{sx}_{lb}_{eh}")
                nc.tensor.transpose(out=tp[:], in_=qloc[lb][:, eh * P:(eh + 1) * P],
                                    identity=ident[:])
                nc.scalar.copy(qlocT[eh][:, lb * P:(lb + 1) * P], tp[:])

        # ---------------- S^T staging (one buffer per corner plane) --------
        stall = [cp.tile([P, 8 * ROWS], f16, name=f"stall{sx}_{pp}")
                 for pp in range(N_PLANES)]

        # ---------------- per-rowtile pipeline ----------------
        for lb in range(4):
            proj_ps = psA.tile([P, 96], f32, space="PSUM", tag="proj_ps",
                               name=f"proj_ps{sx}_{lb}")
            nc.tensor.matmul(proj_ps[:], lhsT=qlocT[0][:, lb * P:(lb + 1) * P],
                             rhs=wcat[0][:], start=True, stop=False)
            nc.tensor.matmul(proj_ps[:], lhsT=qlocT[1][:, lb * P:(lb + 1) * P],
                             rhs=wcat[1][:], start=False, stop=False)
            nc.tensor.matmul(proj_ps[:], lhsT=ones1[:], rhs=bcat[:],
                             start=False, stop=True)
            proj = wp.tile([P, 96], f32, tag="proj", name=f"proj{sx}_{lb}")
            nc.scalar.copy(proj[:], proj_ps[:])

            refx = refs[lb][:, 0:1]
            refy = refs[lb][:, 1:2]

            # pos: x in cols 0:32, y in cols 32:64
            pos = wp.tile([P, 2 * NT], f32, tag="pos", name=f"pos{sx}_{lb}")
            nc.vector.tensor_scalar(pos[:, 0:NT], proj[:, 0:64:2], refx, 32.0,
                                    Alu.add, Alu.mult)
            nc.vector.tensor_scalar(pos[:, NT:2 * NT], proj[:, 1:64:2], refy, 32.0,
                                    Alu.add, Alu.mult)

            # floors (batched x|y): f = rni(pos - 1.0)  [= floor(pos - 0.5)]
            fl = wp.tile([P, 2 * NT], f32, tag="fl", name=f"fl{sx}_{lb}")
            nc.vector.tensor_scalar(fl[:], pos[:], 1.0 - MAGIC, None, Alu.subtract)
            nc.vector.tensor_scalar(fl[:], fl[:], MAGIC, None, Alu.subtract)
            fx = fl[:, 0:NT]
            fy = fl[:, NT:2 * NT]

            # fracs: dd = pos - f; w1 = dd-0.5, w0 = 1.5-dd  (batched x|y)
            dd = wp.tile([P, 2 * NT], f32, tag="dd", name=f"dd{sx}_{lb}")
            nc.vector.tensor_sub(dd[:], pos[:], fl[:])
            w1 = wp.tile([P, 2 * NT], f32, tag="w1", name=f"w1{sx}_{lb}")
            nc.vector.tensor_scalar(w1[:], dd[:], 0.5, None, Alu.subtract)
            w0 = wp.tile([P, 2 * NT], f32, tag="w0", name=f"w0{sx}_{lb}")
            nc.vector.tensor_scalar(w0[:], dd[:], -1.0, 1.5, Alu.mult, Alu.add)
            # wxpair = [wx0 | wx1], wypair = [wy0 | wy1]
            wxpair = wp.tile([P, 2 * NT], f32, tag="wxp", name=f"wxp{sx}_{lb}")
            nc.vector.tensor_copy(wxpair[:, 0:NT], w0[:, 0:NT])
            nc.vector.tensor_copy(wxpair[:, NT:2 * NT], w1[:, 0:NT])
            wypair = wp.tile([P, 2 * NT], f32, tag="wyp", name=f"wyp{sx}_{lb}")
            nc.vector.tensor_copy(wypair[:, 0:NT], w0[:, NT:2 * NT])
            nc.vector.tensor_copy(wypair[:, NT:2 * NT], w1[:, NT:2 * NT])

            # attention softmax over points, /8 folded into reciprocal
            ex = wp.tile([P, NT], f32, tag="ex", name=f"ex{sx}_{lb}")
            nc.scalar.activation(ex[:], proj[:, 64:96], Act.Exp)
            r1 = wp.tile([P, 16], f32, tag="r1", name=f"r1{sx}_{lb}")
            nc.vector.tensor_add(r1[:], ex[:, 0:32:2], ex[:, 1:32:2])
            s4 = wp.tile([P, 8], f32, tag="s4", name=f"s4{sx}_{lb}")
            nc.vector.tensor_add(s4[:], r1[:, 0:16:2], r1[:, 1:16:2])
            rcp = wp.tile([P, 8], f32, tag="rcp", name=f"rcp{sx}_{lb}")
            nc.vector.reciprocal(rcp[:], s4[:])
            nc.vector.tensor_scalar(rcp[:], rcp[:], 0.125, None, Alu.mult)
            a8 = wp.tile([P, NT], f32, tag="a8", name=f"a8{sx}_{lb}")
            nc.vector.tensor_tensor(
                a8[:].rearrange("p (h q) -> p h q", q=4),
                ex[:].rearrange("p (h q) -> p h q", q=4),
                rcp[:].unsqueeze(2).broadcast_to([P, 8, 4]),
                op=Alu.mult)

            # axpair = wxpair * attn/8 (attn replicated over the 2 halves)
            axpair = wp.tile([P, 2 * NT], f32, tag="axp", name=f"axp{sx}_{lb}")
            nc.vector.tensor_tensor(
                axpair[:].rearrange("p (h t) -> p h t", t=NT),
                wxpair[:].rearrange("p (h t) -> p h t", t=NT),
                a8[:].unsqueeze(1).broadcast_to([P, 2, NT]),
                op=Alu.mult)

            # corner weights, class-major [c*32+t], c = cy*2+cx
            wit = wp.tile([P, 4 * NT], f32, tag="wit", name=f"wit{sx}_{lb}")
            nc.vector.tensor_tensor(
                wit[:].rearrange("p (cy cx t) -> p cy cx t", cy=2, cx=2),
                axpair[:].rearrange("p (cx t) -> p cx t", cx=2)
                    .unsqueeze(1).broadcast_to([P, 2, 2, NT]),
                wypair[:].rearrange("p (cy t) -> p cy t", cy=2)
                    .unsqueeze(2).broadcast_to([P, 2, 2, NT]),
                op=Alu.mult)
            wit16 = wp.tile([P, 4 * NT], f16, tag="wit16", name=f"wit16{sx}_{lb}")
            nc.scalar.copy(wit16[:], wit[:])

            # cell = 32*fy + fx ; alias-free key bkey = cell + 96*fy
            cell = wp.tile([P, NT], f32, tag="cell", name=f"cell{sx}_{lb}")
            nc.vector.tensor_scalar(cell[:], fy, 32.0, None, Alu.mult)
            nc.vector.tensor_tensor(cell[:], cell[:], fx, op=Alu.add)
            bkey = wp.tile([P, NT], f32, tag="bkey", name=f"bkey{sx}_{lb}")
            nc.vector.tensor_scalar(bkey[:], fy, 96.0, None, Alu.mult)
            nc.vector.tensor_tensor(bkey[:], bkey[:], cell[:], op=Alu.add)

            # validity (batched x|y): v0 = in [0,31], v1 = in [-1,30]
            v0 = wp.tile([P, 2 * NT], f32, tag="v0", name=f"v0{sx}_{lb}")
            nc.vector.tensor_scalar(v0[:], fl[:], 0.0, None, Alu.is_ge)
            vtmp = wp.tile([P, 2 * NT], f32, tag="vtmp", name=f"vtmp{sx}_{lb}")
            nc.vector.tensor_scalar(vtmp[:], fl[:], 31.0, None, Alu.is_le)
            nc.vector.tensor_tensor(v0[:], v0[:], vtmp[:], op=Alu.mult)
            v1 = wp.tile([P, 2 * NT], f32, tag="v1", name=f"v1{sx}_{lb}")
            nc.vector.tensor_scalar(v1[:], fl[:], -1.0, None, Alu.is_ge)
            nc.vector.tensor_scalar(vtmp[:], fl[:], 30.0, None, Alu.is_le)
            nc.vector.tensor_tensor(v1[:], v1[:], vtmp[:], op=Alu.mult)
            # vxpair = [vx0 | vx1], vypair = [vy0 | vy1]
            vxpair = wp.tile([P, 2 * NT], f32, tag="vxp", name=f"vxp{sx}_{lb}")
            nc.vector.tensor_copy(vxpair[:, 0:NT], v0[:, 0:NT])
            nc.vector.tensor_copy(vxpair[:, NT:2 * NT], v1[:, 0:NT])
            vypair = wp.tile([P, 2 * NT], f32, tag="vyp", name=f"vyp{sx}_{lb}")
            nc.vector.tensor_copy(vypair[:, 0:NT], v0[:, NT:2 * NT])
            nc.vector.tensor_copy(vypair[:, NT:2 * NT], v1[:, NT:2 * NT])

            # --- tap equality matrix (f16 out) + keep mask ---
            eq16 = wp.tile([P, NT * NT], f16, tag="eq16", name=f"eq16{sx}_{lb}")
            nc.vector.tensor_tensor(
                eq16[:].rearrange("p (t s) -> p t s", s=NT),
                bkey[:].unsqueeze(2).broadcast_to([P, NT, NT]),
                bkey[:].unsqueeze(1).broadcast_to([P, NT, NT]),
                op=Alu.is_equal)
            dupp = wp.tile([P, NT * NT], f16, tag="dupp", name=f"dupp{sx}_{lb}")
            nc.vector.tensor_tensor(dupp[:], eq16[:], lt16[:], op=Alu.mult)
            dupf = wp.tile([P, NT], f16, tag="dupf", name=f"dupf{sx}_{lb}")
            with nc.allow_low_precision(reason="0/1 flags, fp32 internal"):
                nc.vector.tensor_reduce(
                    dupf[:].unsqueeze(2),
                    dupp[:].rearrange("p (t s) -> p t s", s=NT),
                    op=Alu.max, axis=mybir.AxisListType.X)
            keep = wp.tile([P, NT], f32, tag="keep", name=f"keep{sx}_{lb}")
            nc.vector.tensor_scalar(keep[:], dupf[:], -1.0, 1.0, Alu.mult, Alu.add)

            # --- merged weights, all 4 classes in one TT + one reduce ---
            prod4 = wp.tile([P, 4 * NT * NT], f16, tag="prod4",
                            name=f"prod4{sx}_{lb}")
            nc.vector.tensor_tensor(
                prod4[:].rearrange("p (c t s) -> p c t s", c=4, t=NT),
                eq16[:].rearrange("p (t s) -> p t s", s=NT)
                    .unsqueeze(1).broadcast_to([P, 4, NT, NT]),
                wit16[:].rearrange("p (c s) -> p c s", c=4)
                    .unsqueeze(2).broadcast_to([P, 4, NT, NT]),
                op=Alu.mult)
            mw = wp.tile([P, 4 * NT], f16, tag="mw", name=f"mw{sx}_{lb}")
            with nc.allow_low_precision(reason="<=6 small f16 addends"):
                nc.vector.tensor_reduce(
                    mw[:].unsqueeze(2),
                    prod4[:].rearrange("p (ct s) -> p ct s", s=NT),
                    op=Alu.add, axis=mybir.AxisListType.X)

            # --- scatter indices: idx = keep*vx*vy*(cell+off_c+...) - 1 ---
            km4 = wp.tile([P, 4 * NT], f32, tag="km4", name=f"km4{sx}_{lb}")
            nc.vector.tensor_tensor(
                km4[:].rearrange("p (cy cx t) -> p cy cx t", cy=2, cx=2),
                vxpair[:].rearrange("p (cx t) -> p cx t", cx=2)
                    .unsqueeze(1).broadcast_to([P, 2, 2, NT]),
                vypair[:].rearrange("p (cy t) -> p cy t", cy=2)
                    .unsqueeze(2).broadcast_to([P, 2, 2, NT]),
                op=Alu.mult)
            nc.vector.tensor_tensor(
                km4[:].rearrange("p (c t) -> p c t", c=4),
                km4[:].rearrange("p (c t) -> p c t", c=4),
                keep[:].unsqueeze(1).broadcast_to([P, 4, NT]),
                op=Alu.mult)
            cell4 = wp.tile([P, 4 * NT], f32, tag="cell4", name=f"cell4{sx}_{lb}")
            nc.vector.tensor_tensor(
                cell4[:].rearrange("p (c t) -> p c t", c=4),
                cell[:].unsqueeze(1).broadcast_to([P, 4, NT]),
                off4[:].rearrange("p (c t) -> p c t", c=4),
                op=Alu.add)
            nc.vector.tensor_tensor(cell4[:], cell4[:], km4[:], op=Alu.mult)
            nc.vector.tensor_scalar(cell4[:], cell4[:], 1.0, None, Alu.subtract)
            idx16 = wp.tile([P, 4 * NT], i16, tag="idx16", name=f"idx16{sx}_{lb}")
            nc.vector.tensor_copy(idx16[:], cell4[:])

            # --- scatters: one fp16 plane per corner class ---
            for c in range(N_PLANES):
                pl = wp.tile([P, L], f16, tag=f"plane{c}",
                             name=f"plane{sx}_{lb}_{c}")
                nc.gpsimd.local_scatter(pl[:], mw[:, c * NT:(c + 1) * NT],
                                        idx16[:, c * NT:(c + 1) * NT],
                                        channels=P, num_elems=L, num_idxs=NT)
                dst = stall[c][:, :].rearrange("p (kt l) -> p kt l", l=ROWS)
                dst = dst[:, :, lb * P:(lb + 1) * P]
                nc.sync.dma_start_transpose(out=dst, in_=pl[:])

        # ---------------- sampling matmul (planes folded into K) ----------
        outT16 = []
        for eh in range(2):
            ps = psB.tile([P, ROWS], f32, space="PSUM", tag="outT_ps",
                          name=f"outT_ps{sx}_{eh}")
            first = True
            for pp in range(N_PLANES):
                st3 = stall[pp][:, :].rearrange("p (kt l) -> p kt l", l=ROWS)
                for kt in range(8):
                    nc.tensor.matmul(ps[:],
                                     lhsT=q16[kt][:, eh * P:(eh + 1) * P],
                                     rhs=st3[:, kt, :],
                                     start=first,
                                     stop=(pp == N_PLANES - 1 and kt == 7))
                    first = False
            o16 = cp.tile([P, ROWS], f16, name=f"outT16{sx}_{eh}")
            nc.vector.tensor_copy(o16[:], ps[:])
            outT16.append(o16)

        # ---------------- final projection ----------------
        out3 = d["out"].ap().rearrange("(t p) e -> t p e", p=P)
        for lb in range(4):
            fin = psA.tile([P, E], f32, space="PSUM", tag="fin_ps",
                           name=f"fin_ps{sx}_{lb}")
            nc.tensor.matmul(fin[:], lhsT=outT16[0][:, lb * P:(lb + 1) * P],
                             rhs=wout16[0][:], start=True, stop=False)
            nc.tensor.matmul(fin[:], lhsT=outT16[1][:, lb * P:(lb + 1) * P],
                             rhs=wout16[1][:], start=False, stop=False)
            nc.tensor.matmul(fin[:], lhsT=ones1h[:], rhs=bout16[:],
                             start=False, stop=True)
            osb = wp.tile([P, E], f32, tag="osb", name=f"osb{sx}_{lb}")
            nc.scalar.copy(osb[:], fin[:])
            nc.sync.dma_start(out3[lb], osb[:])


def build_program(repeat=1, strip=True):
    nc = bacc.Bacc("TRN2", target_bir_lowering=False, debug=False)

    d = {
        "qn": nc.dram_tensor("qn", [L, E], f32, kind="ExternalInput"),
        "qloc": nc.dram_tensor("qloc", [ROWS, E], f32, kind="ExternalInput"),
        "refs": nc.dram_tensor("refs", [ROWS, 2], f32, kind="ExternalInput"),
        "wcat": nc.dram_tensor("wcat", [E, 96], f32, kind="ExternalInput"),
        "wout": nc.dram_tensor("wout", [E, E], f32, kind="ExternalInput"),
        "bcat": nc.dram_tensor("bcat", [1, 96], f32, kind="ExternalInput"),
        "bout": nc.dram_tensor("bout", [1, E], f32, kind="ExternalInput"),
        "out": nc.dram_tensor("out", [ROWS, E], f32, kind="ExternalOutput"),
    }

    with tile.TileContext(nc) as tc:
        if repeat == 1:
            _emit(nc, tc, d, "")
        else:
            with tc.For_i(0, repeat, 1):
                _emit(nc, tc, d, "")

    nc.compile()
    if strip:
        nc.m = get_hw_module(nc.m)
    return nc


_NC = None


def _get_nc():
    global _NC
    if _NC is None:
        _NC = build_program()
    return _NC


def make_in_maps(inputs):
    query = np.asarray(inputs["query"], np.float32)
    refp = np.asarray(inputs["reference_points"], np.float32)
    W_off = np.asarray(inputs["W_off"], np.float32)
    b_off = np.asarray(inputs["b_off"], np.float32)
    W_attn = np.asarray(inputs["W_attn"], np.float32)
    b_attn = np.asarray(inputs["b_attn"], np.float32)
    W_out = np.asarray(inputs["W_out"], np.float32)
    b_out = np.asarray(inputs["b_out"], np.float32)

    N = query.shape[0]
    q = query.reshape(N, L, E)
    wcat = np.ascontiguousarray(np.concatenate([W_off, W_attn], axis=1))
    bcat = np.ascontiguousarray(np.concatenate([b_off, b_attn])[None, :])
    bout = np.ascontiguousarray(b_out[None, :])

    in_maps = []
    for c in range(N_CORES):
        n, half = c // 2, c % 2
        lo = half * ROWS
        in_maps.append({
            "qn": np.ascontiguousarray(q[n]),
            "qloc": np.ascontiguousarray(q[n, lo:lo + ROWS]),
            "refs": np.ascontiguousarray(refp[n, lo:lo + ROWS]),
            "wcat": wcat,
            "wout": np.ascontiguousarray(W_out),
            "bcat": bcat,
            "bout": bout,
        })
    return in_maps


def kernel(**inputs):
    nc = _get_nc()
    in_maps = make_in_maps(inputs)
    res = run_bass_kernel_spmd(nc, in_maps, list(range(N_CORES)))
    N = np.asarray(inputs["query"]).shape[0]
    out = np.empty((N, L, E), np.float32)
    for c in range(N_CORES):
        n, half = c // 2, c % 2
        out[n, half * ROWS:(half + 1) * ROWS] = res.results[c]["out"]
    return out.reshape(N, 32, 32, E)


# revision 25
# speedup vs baseline: 1.5660x; 1.1294x over previous
"""Trainium2 Bass kernel for MultiHeadDeformableAttention2D.

Strategy (8 cores, SPMD): data-parallel over (batch n, row-half). Core c
handles n = c//2 and 512 query rows. The deformable sampling is cast as a
sparse matrix multiply: out_pre = S @ q where S[l, cell] accumulates
attn*bilinear weights. S is built with per-partition GPSIMD local_scatter
into 4 corner-class planes; same-base-cell duplicates within a class are
merged exactly beforehand via a 32x32 tap-equality matrix on the vector
engine. The contraction with q and the output projection W_out run on the
tensor engine in fp16 with fp32 accumulation; the 4 planes are folded into
the contraction (PSUM accumulation) so no plane-combine pass is needed.
"""

import sys

sys.path.insert(0, "/opt/trn_rl_repo")

import numpy as np

import concourse.bass as bass  # noqa: F401
import concourse.tile as tile
from concourse import bacc, mybir
from concourse.bass_interp import get_hw_module
from concourse.bass_utils import run_bass_kernel_spmd
from concourse.masks import make_identity

P = 128
L = 1024          # spatial cells (32*32)
E = 256           # embed dim
NT = 32           # taps = heads(8) * points(4)
ROWS = 512        # rows per core
N_CORES = 8
N_PLANES = 1      # 1, 2, or 4: planes k-folded into the PE contraction
MAGIC = 12582912.0  # 1.5*2^23; (x + MAGIC) - MAGIC rounds x to nearest int
ABLATE = set()    # {"merge", "scatter", "smalls", "sample", "dup"} for timing ablations

f32 = mybir.dt.float32
f16 = mybir.dt.float16
i16 = mybir.dt.int16
i32 = mybir.dt.int32
Alu = mybir.AluOpType
Act = mybir.ActivationFunctionType


def _emit(nc, tc, d, sx):
    """Emit one full kernel pass. d: dict of dram tensors, sx: name suffix."""
    with (
        tc.tile_pool(name=f"const{sx}", bufs=1) as cp,
        tc.tile_pool(name=f"work{sx}", bufs=3) as wp,
        tc.tile_pool(name=f"psA{sx}", bufs=2, space="PSUM") as psA,
        tc.tile_pool(name=f"psB{sx}", bufs=2, space="PSUM") as psB,
    ):
        # ---------------- constants ----------------
        ident = cp.tile([P, P], f32, name=f"ident{sx}")
        make_identity(nc, ident[:])
        ones1 = cp.tile([1, P], f32, name=f"ones1{sx}")
        nc.vector.memset(ones1[:], 1.0)
        ones1h = cp.tile([1, P], f16, name=f"ones1h{sx}")
        nc.vector.memset(ones1h[:], 1.0)

        # strict-lower mask lt16[t*32+s] = 1.0 iff s < t
        lt_i = cp.tile([P, NT * NT], i32, name=f"lt_i{sx}")
        nc.gpsimd.iota(lt_i[:], pattern=[[-1, NT], [1, NT]], base=0,
                       channel_multiplier=0)
        lt16 = cp.tile([P, NT * NT], f16, name=f"lt16{sx}")
        nc.vector.tensor_scalar(lt16[:], lt_i[:], -1.0, None, Alu.is_le)

        # per-class cell offsets (+1 for the idx trick), class-major [c*32+t]
        off4 = cp.tile([P, 4 * NT], f32, name=f"off4{sx}")
        for c, off in enumerate((1.0, 2.0, 33.0, 34.0)):
            nc.vector.memset(off4[:, c * NT:(c + 1) * NT], off)

        # ---------------- loads ----------------
        qn3 = d["qn"].ap().rearrange("(t p) e -> t p e", p=P)
        qloc3 = d["qloc"].ap().rearrange("(t p) e -> t p e", p=P)
        refs3 = d["refs"].ap().rearrange("(t p) c -> t p c", p=P)

        # small latency-critical loads first; the 1MB qn load follows
        qloc = []
        for i in range(4):
            t = cp.tile([P, E], f32, name=f"qloc{sx}_{i}")
            nc.sync.dma_start(t[:], qloc3[i])
            qloc.append(t)

        refs = []
        for i in range(4):
            t = cp.tile([P, 2], f32, name=f"refs{sx}_{i}")
            nc.sync.dma_start(t[:], refs3[i])
            refs.append(t)

        wcat3 = d["wcat"].ap().rearrange("(t p) c -> t p c", p=P)
        wcat = []
        for i in range(2):
            t = cp.tile([P, 96], f32, name=f"wcat{sx}_{i}")
            nc.sync.dma_start(t[:], wcat3[i])
            wcat.append(t)

        bcat = cp.tile([1, 96], f32, name=f"bcat{sx}")
        nc.sync.dma_start(bcat[:], d["bcat"].ap()[:])

        q16 = []
        for i in range(8):
            qt = wp.tile([P, E], f32, tag="qload", name=f"qload{sx}_{i}")
            nc.sync.dma_start(qt[:], qn3[i])
            qh = cp.tile([P, E], f16, name=f"q16{sx}_{i}")
            nc.scalar.copy(qh[:], qt[:])
            q16.append(qh)

        wout3 = d["wout"].ap().rearrange("(t p) c -> t p c", p=P)
        wout16 = []
        for i in range(2):
            t = wp.tile([P, E], f32, tag="wo_load", name=f"wo_load{sx}_{i}")
            nc.sync.dma_start(t[:], wout3[i])
            th = cp.tile([P, E], f16, name=f"wout16{sx}_{i}")
            nc.scalar.copy(th[:], t[:])
            wout16.append(th)

        bout = cp.tile([1, E], f32, name=f"bout{sx}")
        nc.sync.dma_start(bout[:], d["bout"].ap()[:])
        bout16 = cp.tile([1, E], f16, name=f"bout16{sx}")
        nc.scalar.copy(bout16[:], bout[:])

        # ---------------- q_loc^T (for projections) ----------------
        qlocT = [cp.tile([P, ROWS], f32, name=f"qlocT{sx}_{eh}") for eh in range(2)]
        for lb in range(4):
            for eh in range(2):
                tp = psA.tile([P, P], f32, space="PSUM", tag="tp_ps",
                              name=f"tp_ps{sx}_{lb}_{eh}")
                nc.tensor.transpose(out=tp[:], in_=qloc[lb][:, eh * P:(eh + 1) * P],
                                    identity=ident[:])
                nc.scalar.copy(qlocT[eh][:, lb * P:(lb + 1) * P], tp[:])

        # ---------------- S^T staging (one buffer per corner plane) --------
        stall = [cp.tile([P, 8 * ROWS], f16, name=f"stall{sx}_{pp}")
                 for pp in range(N_PLANES)]
        if "transp" in ABLATE:
            for st in stall:
                nc.vector.memset(st[:], 0.0)

        # ---------------- projections (all rowtiles up front) -------------
        projs = []
        for lb in range(4):
            proj_ps = psA.tile([P, 96], f32, space="PSUM", tag="proj_ps",
                               name=f"proj_ps{sx}_{lb}")
            nc.tensor.matmul(proj_ps[:], lhsT=qlocT[0][:, lb * P:(lb + 1) * P],
                             rhs=wcat[0][:], start=True, stop=False)
            nc.tensor.matmul(proj_ps[:], lhsT=qlocT[1][:, lb * P:(lb + 1) * P],
                             rhs=wcat[1][:], start=False, stop=False)
            nc.tensor.matmul(proj_ps[:], lhsT=ones1[:], rhs=bcat[:],
                             start=False, stop=True)
            proj = cp.tile([P, 96], name=f"projv{sx}_{lb}", dtype=f32)
            nc.scalar.copy(proj[:], proj_ps[:])
            projs.append(proj)

        # ---------------- per-rowtile pipeline ----------------
        outT16 = [cp.tile([P, ROWS], f16, name=f"outT16{sx}_{eh}")
                  for eh in range(2)]
        out3 = d["out"].ap().rearrange("(t p) e -> t p e", p=P)
        for lb in range(4):
            proj = projs[lb]
            refx = refs[lb][:, 0:1]
            refy = refs[lb][:, 1:2]

            # pos: x in cols 0:32, y in cols 32:64
            pos = wp.tile([P, 2 * NT], f32, tag="pos", name=f"pos{sx}_{lb}")
            nc.vector.tensor_scalar(pos[:, 0:NT], proj[:, 0:64:2], refx, 32.0,
                                    Alu.add, Alu.mult)
            nc.vector.tensor_scalar(pos[:, NT:2 * NT], proj[:, 1:64:2], refy, 32.0,
                                    Alu.add, Alu.mult)

            # floors (batched x|y): f = rni(pos - 1.0)  [= floor(pos - 0.5)]
            fl = wp.tile([P, 2 * NT], f32, tag="fl", name=f"fl{sx}_{lb}")
            nc.vector.tensor_scalar(fl[:], pos[:], 1.0 - MAGIC, None, Alu.subtract)
            nc.vector.tensor_scalar(fl[:], fl[:], MAGIC, None, Alu.subtract)
            fx = fl[:, 0:NT]
            fy = fl[:, NT:2 * NT]

            # fracs: dd = pos - f; w1 = dd-0.5, w0 = 1.5-dd  (batched x|y)
            dd = wp.tile([P, 2 * NT], f32, tag="dd", name=f"dd{sx}_{lb}")
            nc.vector.tensor_sub(dd[:], pos[:], fl[:])
            w1 = wp.tile([P, 2 * NT], f32, tag="w1", name=f"w1{sx}_{lb}")
            nc.vector.tensor_scalar(w1[:], dd[:], 0.5, None, Alu.subtract)
            w0 = wp.tile([P, 2 * NT], f32, tag="w0", name=f"w0{sx}_{lb}")
            nc.vector.tensor_scalar(w0[:], dd[:], -1.0, 1.5, Alu.mult, Alu.add)
            # wxpair = [wx0 | wx1], wypair = [wy0 | wy1]
            wxpair = wp.tile([P, 2 * NT], f32, tag="wxp", name=f"wxp{sx}_{lb}")
            nc.vector.tensor_copy(wxpair[:, 0:NT], w0[:, 0:NT])
            nc.vector.tensor_copy(wxpair[:, NT:2 * NT], w1[:, 0:NT])
            wypair = wp.tile([P, 2 * NT], f32, tag="wyp", name=f"wyp{sx}_{lb}")
            nc.vector.tensor_copy(wypair[:, 0:NT], w0[:, NT:2 * NT])
            nc.vector.tensor_copy(wypair[:, NT:2 * NT], w1[:, NT:2 * NT])

            # attention softmax over points, /8 folded into reciprocal
            ex = wp.tile([P, NT], f32, tag="ex", name=f"ex{sx}_{lb}")
            nc.scalar.activation(ex[:], proj[:, 64:96], Act.Exp)
            r1 = wp.tile([P, 16], f32, tag="r1", name=f"r1{sx}_{lb}")
            nc.vector.tensor_add(r1[:], ex[:, 0:32:2], ex[:, 1:32:2])
            s4 = wp.tile([P, 8], f32, tag="s4", name=f"s4{sx}_{lb}")
            nc.vector.tensor_add(s4[:], r1[:, 0:16:2], r1[:, 1:16:2])
            rcp = wp.tile([P, 8], f32, tag="rcp", name=f"rcp{sx}_{lb}")
            nc.vector.reciprocal(rcp[:], s4[:])
            nc.vector.tensor_scalar(rcp[:], rcp[:], 0.125, None, Alu.mult)
            a8 = wp.tile([P, NT], f32, tag="a8", name=f"a8{sx}_{lb}")
            nc.vector.tensor_tensor(
                a8[:].rearrange("p (h q) -> p h q", q=4),
                ex[:].rearrange("p (h q) -> p h q", q=4),
                rcp[:].unsqueeze(2).broadcast_to([P, 8, 4]),
                op=Alu.mult)

            # axpair = wxpair * attn/8 (attn replicated over the 2 halves)
            axpair = wp.tile([P, 2 * NT], f32, tag="axp", name=f"axp{sx}_{lb}")
            nc.vector.tensor_tensor(
                axpair[:].rearrange("p (h t) -> p h t", t=NT),
                wxpair[:].rearrange("p (h t) -> p h t", t=NT),
                a8[:].unsqueeze(1).broadcast_to([P, 2, NT]),
                op=Alu.mult)

            # corner weights, class-major [c*32+t], c = cy*2+cx
            wit = wp.tile([P, 4 * NT], f32, tag="wit", name=f"wit{sx}_{lb}")
            nc.vector.tensor_tensor(
                wit[:].rearrange("p (cy cx t) -> p cy cx t", cy=2, cx=2),
                axpair[:].rearrange("p (cx t) -> p cx t", cx=2)
                    .unsqueeze(1).broadcast_to([P, 2, 2, NT]),
                wypair[:].rearrange("p (cy t) -> p cy t", cy=2)
                    .unsqueeze(2).broadcast_to([P, 2, 2, NT]),
                op=Alu.mult)
            wit16 = wp.tile([P, 4 * NT], f16, tag="wit16", name=f"wit16{sx}_{lb}")
            nc.scalar.copy(wit16[:], wit[:])

            # cell = 32*fy + fx ; alias-free key bkey = cell + 96*fy
            cell = wp.tile([P, NT], f32, tag="cell", name=f"cell{sx}_{lb}")
            nc.vector.tensor_scalar(cell[:], fy, 32.0, None, Alu.mult)
            nc.vector.tensor_tensor(cell[:], cell[:], fx, op=Alu.add)
            bkey = wp.tile([P, NT], f32, tag="bkey", name=f"bkey{sx}_{lb}")
            nc.vector.tensor_scalar(bkey[:], fy, 96.0, None, Alu.mult)
            nc.vector.tensor_tensor(bkey[:], bkey[:], cell[:], op=Alu.add)

            # validity (batched x|y): v0 = in [0,31], v1 = in [-1,30]
            v0 = wp.tile([P, 2 * NT], f32, tag="v0", name=f"v0{sx}_{lb}")
            nc.vector.tensor_scalar(v0[:], fl[:], 0.0, None, Alu.is_ge)
            vtmp = wp.tile([P, 2 * NT], f32, tag="vtmp", name=f"vtmp{sx}_{lb}")
            nc.vector.tensor_scalar(vtmp[:], fl[:], 31.0, None, Alu.is_le)
            nc.vector.tensor_tensor(v0[:], v0[:], vtmp[:], op=Alu.mult)
            v1 = wp.tile([P, 2 * NT], f32, tag="v1", name=f"v1{sx}_{lb}")
            nc.vector.tensor_scalar(v1[:], fl[:], -1.0, None, Alu.is_ge)
            nc.vector.tensor_scalar(vtmp[:], fl[:], 30.0, None, Alu.is_le)
            nc.vector.tensor_tensor(v1[:], v1[:], vtmp[:], op=Alu.mult)
            # vxpair = [vx0 | vx1], vypair = [vy0 | vy1]
            vxpair = wp.tile([P, 2 * NT], f32, tag="vxp", name=f"vxp{sx}_{lb}")
            nc.vector.tensor_copy(vxpair[:, 0:NT], v0[:, 0:NT])
            nc.vector.tensor_copy(vxpair[:, NT:2 * NT], v1[:, 0:NT])
            vypair = wp.tile([P, 2 * NT], f32, tag="vyp", name=f"vyp{sx}_{lb}")
            nc.vector.tensor_copy(vypair[:, 0:NT], v0[:, NT:2 * NT])
            nc.vector.tensor_copy(vypair[:, NT:2 * NT], v1[:, NT:2 * NT])

            # --- tap equality matrix (f16 out) + keep mask ---
            do_merge = "merge" not in ABLATE
            eq16 = wp.tile([P, NT * NT], f16, tag="eq16", name=f"eq16{sx}_{lb}")
            if not do_merge:
                mw = wit16
                keep = None
            el_ = None
            if do_merge:
              nc.vector.tensor_tensor(
                eq16[:].rearrange("p (t s) -> p t s", s=NT),
                bkey[:].unsqueeze(2).broadcast_to([P, NT, NT]),
                bkey[:].unsqueeze(1).broadcast_to([P, NT, NT]),
                op=Alu.is_equal)
            if do_merge:
              dupp = wp.tile([P, NT * NT], f16, tag="dupp", name=f"dupp{sx}_{lb}")
              nc.vector.tensor_tensor(dupp[:], eq16[:], lt16[:], op=Alu.mult)
              dupf = wp.tile([P, NT], f16, tag="dupf", name=f"dupf{sx}_{lb}")
              with nc.allow_low_precision(reason="0/1 flags, fp32 internal"):
                nc.vector.tensor_reduce(
                    dupf[:].unsqueeze(2),
                    dupp[:].rearrange("p (t s) -> p t s", s=NT),
                    op=Alu.max, axis=mybir.AxisListType.X)
              keep = wp.tile([P, NT], f32, tag="keep", name=f"keep{sx}_{lb}")
              nc.vector.tensor_scalar(keep[:], dupf[:], -1.0, 1.0, Alu.mult, Alu.add)

            # --- merged weights, all 4 classes in one TT + one reduce ---
            if do_merge:
              prod4 = wp.tile([P, 4 * NT * NT], f16, tag="prod4",
                            name=f"prod4{sx}_{lb}")
              nc.vector.tensor_tensor(
                prod4[:].rearrange("p (c t s) -> p c t s", c=4, t=NT),
                eq16[:].rearrange("p (t s) -> p t s", s=NT)
                    .unsqueeze(1).broadcast_to([P, 4, NT, NT]),
                wit16[:].rearrange("p (c s) -> p c s", c=4)
                    .unsqueeze(2).broadcast_to([P, 4, NT, NT]),
                op=Alu.mult)
              # log-tree segmented sum over s (5 halving adds, all f16 2x)
              tr = prod4[:].rearrange("p (ct s) -> p ct s", s=NT)
              t16a = wp.tile([P, 4 * NT * 16], f16, tag="t16a",
                             name=f"t16a{sx}_{lb}")
              a_v = t16a[:].rearrange("p (ct s) -> p ct s", s=16)
              nc.vector.tensor_tensor(a_v, tr[:, :, 0:16], tr[:, :, 16:32],
                                      op=Alu.add)
              t8 = wp.tile([P, 4 * NT * 8], f16, tag="t8", name=f"t8{sx}_{lb}")
              b_v = t8[:].rearrange("p (ct s) -> p ct s", s=8)
              nc.vector.tensor_tensor(b_v, a_v[:, :, 0:8], a_v[:, :, 8:16],
                                      op=Alu.add)
              t4 = wp.tile([P, 4 * NT * 4], f16, tag="t4", name=f"t4{sx}_{lb}")
              c_v = t4[:].rearrange("p (ct s) -> p ct s", s=4)
              nc.vector.tensor_tensor(c_v, b_v[:, :, 0:4], b_v[:, :, 4:8],
                                      op=Alu.add)
              t2 = wp.tile([P, 4 * NT * 2], f16, tag="t2", name=f"t2{sx}_{lb}")
              d_v = t2[:].rearrange("p (ct s) -> p ct s", s=2)
              nc.vector.tensor_tensor(d_v, c_v[:, :, 0:2], c_v[:, :, 2:4],
                                      op=Alu.add)
              mw = wp.tile([P, 4 * NT], f16, tag="mw", name=f"mw{sx}_{lb}")
              nc.vector.tensor_tensor(mw[:].unsqueeze(2), d_v[:, :, 0:1],
                                      d_v[:, :, 1:2], op=Alu.add)

            # --- scatter indices: idx = keep*vx*vy*(cell+off_c+...) - 1 ---
            km4 = wp.tile([P, 4 * NT], f32, tag="km4", name=f"km4{sx}_{lb}")
            nc.vector.tensor_tensor(
                km4[:].rearrange("p (cy cx t) -> p cy cx t", cy=2, cx=2),
                vxpair[:].rearrange("p (cx t) -> p cx t", cx=2)
                    .unsqueeze(1).broadcast_to([P, 2, 2, NT]),
                vypair[:].rearrange("p (cy t) -> p cy t", cy=2)
                    .unsqueeze(2).broadcast_to([P, 2, 2, NT]),
                op=Alu.mult)
            if keep is not None:
                nc.vector.tensor_tensor(
                    km4[:].rearrange("p (c t) -> p c t", c=4),
                    km4[:].rearrange("p (c t) -> p c t", c=4),
                    keep[:].unsqueeze(1).broadcast_to([P, 4, NT]),
                    op=Alu.mult)
            cell4 = wp.tile([P, 4 * NT], f32, tag="cell4", name=f"cell4{sx}_{lb}")
            nc.vector.tensor_tensor(
                cell4[:].rearrange("p (c t) -> p c t", c=4),
                cell[:].unsqueeze(1).broadcast_to([P, 4, NT]),
                off4[:].rearrange("p (c t) -> p c t", c=4),
                op=Alu.add)
            nc.vector.tensor_tensor(cell4[:], cell4[:], km4[:], op=Alu.mult)
            nc.vector.tensor_scalar(cell4[:], cell4[:], 1.0, None, Alu.subtract)
            idx16 = wp.tile([P, 4 * NT], i16, tag="idx16", name=f"idx16{sx}_{lb}")
            nc.vector.tensor_copy(idx16[:], cell4[:])

            # --- scatters: one fp16 plane per corner class, then combine ---
            planes = []
            for c in range(4):
                pl = wp.tile([P, L], f16, tag=f"plane{c}",
                             name=f"plane{sx}_{lb}_{c}")
                if "scatter" in ABLATE:
                    nc.vector.tensor_copy(pl[:, 0:4 * NT], mw[:])
                else:
                    nc.gpsimd.local_scatter(pl[:], mw[:, c * NT:(c + 1) * NT],
                                            idx16[:, c * NT:(c + 1) * NT],
                                            channels=P, num_elems=L, num_idxs=NT)
                planes.append(pl)
            if N_PLANES == 1:
                s01 = wp.tile([P, L], f16, tag="s01", name=f"s01{sx}_{lb}")
                nc.vector.tensor_add(s01[:], planes[0][:], planes[1][:])
                s23 = wp.tile([P, L], f16, tag="s23", name=f"s23{sx}_{lb}")
                nc.vector.tensor_add(s23[:], planes[2][:], planes[3][:])
                sall = wp.tile([P, L], f16, tag="sall", name=f"sall{sx}_{lb}")
                nc.vector.tensor_add(sall[:], s01[:], s23[:])
                srcs = [sall]
            elif N_PLANES == 2:
                s01 = wp.tile([P, L], f16, tag="s01", name=f"s01{sx}_{lb}")
                nc.vector.tensor_add(s01[:], planes[0][:], planes[1][:])
                s23 = wp.tile([P, L], f16, tag="s23", name=f"s23{sx}_{lb}")
                nc.vector.tensor_add(s23[:], planes[2][:], planes[3][:])
                srcs = [s01, s23]
            else:
                srcs = planes
            if "transp" not in ABLATE:
                for pp, s_src in enumerate(srcs):
                    dst = stall[pp][:, :].rearrange("p (kt l) -> p kt l", l=ROWS)
                    dst = dst[:, :, lb * P:(lb + 1) * P]
                    nc.sync.dma_start_transpose(out=dst, in_=s_src[:])

            # --- sampling for this l-block (starts as soon as S^T lands) ---
            ps = psB.tile([P, 2 * P], f32, space="PSUM", tag="outT_ps",
                          name=f"outT_ps{sx}_{lb}")
            for eh in range(2):
                first = True
                for pp in range(len(srcs)):
                    st3 = stall[pp][:, :].rearrange("p (kt l) -> p kt l", l=ROWS)
                    for kt in range(8):
                        if "sample" in ABLATE and kt > 0:
                            continue
                        nc.tensor.matmul(ps[:, eh * P:(eh + 1) * P],
                                         lhsT=q16[kt][:, eh * P:(eh + 1) * P],
                                         rhs=st3[:, kt, lb * P:(lb + 1) * P],
                                         start=first,
                                         stop=("sample" in ABLATE) or
                                              (pp == len(srcs) - 1 and kt == 7))
                        first = False
                nc.vector.tensor_copy(outT16[eh][:, lb * P:(lb + 1) * P],
                                      ps[:, eh * P:(eh + 1) * P])

            # --- final projection for this l-block ---
            fin = psA.tile([P, E], f32, space="PSUM", tag="fin_ps",
                           name=f"fin_ps{sx}_{lb}")
            nc.tensor.matmul(fin[:], lhsT=outT16[0][:, lb * P:(lb + 1) * P],
                             rhs=wout16[0][:], start=True, stop=False)
            nc.tensor.matmul(fin[:], lhsT=outT16[1][:, lb * P:(lb + 1) * P],
                             rhs=wout16[1][:], start=False, stop=False)
            nc.tensor.matmul(fin[:], lhsT=ones1h[:], rhs=bout16[:],
                             start=False, stop=True)
            osb = wp.tile([P, E], f32, tag="osb", name=f"osb{sx}_{lb}")
            nc.scalar.copy(osb[:], fin[:])
            nc.sync.dma_start(out3[lb], osb[:])


def build_program(repeat=1, strip=True):
    nc = bacc.Bacc("TRN2", target_bir_lowering=False, debug=False)

    d = {
        "qn": nc.dram_tensor("qn", [L, E], f32, kind="ExternalInput"),
        "qloc": nc.dram_tensor("qloc", [ROWS, E], f32, kind="ExternalInput"),
        "refs": nc.dram_tensor("refs", [ROWS, 2], f32, kind="ExternalInput"),
        "wcat": nc.dram_tensor("wcat", [E, 96], f32, kind="ExternalInput"),
        "wout": nc.dram_tensor("wout", [E, E], f32, kind="ExternalInput"),
        "bcat": nc.dram_tensor("bcat", [1, 96], f32, kind="ExternalInput"),
        "bout": nc.dram_tensor("bout", [1, E], f32, kind="ExternalInput"),
        "out": nc.dram_tensor("out", [ROWS, E], f32, kind="ExternalOutput"),
    }

    with tile.TileContext(nc) as tc:
        if repeat == 1:
            _emit(nc, tc, d, "")
        else:
            with tc.For_i(0, repeat, 1):
                _emit(nc, tc, d, "")

    nc.compile()
    if strip:
        nc.m = get_hw_module(nc.m)
    return nc


_NC = None


def _get_nc():
    global _NC
    if _NC is None:
        _NC = build_program()
    return _NC


def make_in_maps(inputs):
    query = np.asarray(inputs["query"], np.float32)
    refp = np.asarray(inputs["reference_points"], np.float32)
    W_off = np.asarray(inputs["W_off"], np.float32)
    b_off = np.asarray(inputs["b_off"], np.float32)
    W_attn = np.asarray(inputs["W_attn"], np.float32)
    b_attn = np.asarray(inputs["b_attn"], np.float32)
    W_out = np.asarray(inputs["W_out"], np.float32)
    b_out = np.asarray(inputs["b_out"], np.float32)

    N = query.shape[0]
    q = query.reshape(N, L, E)
    wcat = np.ascontiguousarray(np.concatenate([W_off, W_attn], axis=1))
    bcat = np.ascontiguousarray(np.concatenate([b_off, b_attn])[None, :])
    bout = np.ascontiguousarray(b_out[None, :])

    in_maps = []
    for c in range(N_CORES):
        n, half = c // 2, c % 2
        lo = half * ROWS
        in_maps.append({
            "qn": np.ascontiguousarray(q[n]),
            "qloc": np.ascontiguousarray(q[n, lo:lo + ROWS]),
            "refs": np.ascontiguousarray(refp[n, lo:lo + ROWS]),
            "wcat": wcat,
            "wout": np.ascontiguousarray(W_out),
            "bcat": bcat,
            "bout": bout,
        })
    return in_maps


def kernel(**inputs):
    nc = _get_nc()
    in_maps = make_in_maps(inputs)
    res = run_bass_kernel_spmd(nc, in_maps, list(range(N_CORES)))
    N = np.asarray(inputs["query"]).shape[0]
    out = np.empty((N, L, E), np.float32)
    for c in range(N_CORES):
        n, half = c // 2, c % 2
        out[n, half * ROWS:(half + 1) * ROWS] = res.results[c]["out"]
    return out.reshape(N, 32, 32, E)
